# revision 6
# baseline (speedup 1.0000x reference)
"""Multi-head causal self-attention with RoPE for Trainium2 (8 NeuronCores).

Problem: B=4, T=2048, C=1024, H=16 heads, D=64, fused QKV + causal softmax
attention + out-projection, fp32 I/O.

Sharding (Megatron-style): core c -> batch b = c//2, heads [8*(c%2), +8).
Each core computes its 8 heads' attention for its batch and a row-parallel
partial of the out-projection; the host sums the two partials per batch.

All matmuls run as float32r (TF32-like, full PE rate, ~FP22 precision).

Per-core kernel phases:
  0. PE-transpose x [T,C] -> xT [C,T], staged through DRAM scratch.
  1. QKV projections: q^T,k^T in [d,t] layout (head dims permuted into
     even/odd groups of 4 heads for RoPE), v in natural [t,d] layout with a
     ones-column appended (gives softmax denominators for free).  RoPE applied
     on-chip to q^T,k^T.
  2. Attention per (512-query chunk j, 4-head group): scores S^T[k,q] via
     4-way row-packed K=32 matmuls, exp on ScalarE, causal mask multiply,
     attn@V accumulated over key tiles in PSUM.  Softmax normalization via
     reciprocal + GpSimd partition-broadcast.
  3. Out-projection per chunk with W_out rows for this core's heads.
"""

import numpy as np

B, T, C = 4, 2048, 1024
H, D = 16, 64
HC = 8               # heads per core
N_CORES = 8
THETA = 10000.0
NJ = T // 512        # 4 query/column chunks
NKT = T // 128       # 16 key tiles
NCT = C // 128       # 8 contraction tiles for projections

_CACHE = {}


def _build_program():
    import concourse.tile as tile
    import concourse.mybir as mybir
    from concourse import bacc

    f32 = mybir.dt.float32
    f32r = mybir.dt.float32r
    EXP = mybir.ActivationFunctionType.Exp
    MUL = mybir.AluOpType.mult
    SUB = mybir.AluOpType.subtract
    ADD = mybir.AluOpType.add

    nc = bacc.Bacc("TRN2", target_bir_lowering=False, debug=False)
    x_t = nc.dram_tensor("x", [T, C], f32r, kind="ExternalInput")
    wqk_t = nc.dram_tensor("wqk", [C, 2 * HC * D], f32r, kind="ExternalInput")
    wv_t = nc.dram_tensor("wv", [C, HC * D], f32r, kind="ExternalInput")
    wout_t = nc.dram_tensor("wout", [HC * D, C], f32r, kind="ExternalInput")
    cs_t = nc.dram_tensor("cs", [128, T], f32r, kind="ExternalInput")
    sn_t = nc.dram_tensor("sn", [128, T], f32r, kind="ExternalInput")
    mk_t = nc.dram_tensor("mk", [128, 1024], f32r, kind="ExternalInput")
    id_t = nc.dram_tensor("ident", [128, 128], f32r, kind="ExternalInput")
    y_t = nc.dram_tensor("y", [T, C], f32, kind="ExternalOutput")

    with tile.TileContext(nc) as tc:
        import contextlib
        with contextlib.ExitStack() as ctx:
            singles = ctx.enter_context(tc.tile_pool(name="singles", bufs=1))
            dram = ctx.enter_context(tc.tile_pool(name="dram", bufs=1, space="DRAM"))
            psum = ctx.enter_context(tc.tile_pool(name="psum", bufs=1, space="PSUM"))
            work = ctx.enter_context(tc.tile_pool(name="work", bufs=1))

            # ---- resident tensors -------------------------------------------
            kT_sb = singles.tile([128, 4, T], f32r, name="kT_sb")
            v_sb = singles.tile([128, NKT, HC, D + 1], f32r, name="v_sb")
            wv_sb = singles.tile([128, NCT, HC * D], f32r, name="wv_sb")
            wout_sb = singles.tile([128, 4, C], f32r, name="wout_sb")
            mk_sb = singles.tile([128, 1024], f32r, name="mk_sb")
            id_sb = singles.tile([128, 128], f32r, name="id_sb")

            nc.sync.dma_start(wv_sb[:], wv_t.ap().rearrange("(kt p) n -> p kt n", p=128))
            nc.sync.dma_start(wout_sb[:], wout_t.ap().rearrange("(ct p) n -> p ct n", p=128))
            nc.sync.dma_start(mk_sb[:], mk_t.ap())
            nc.sync.dma_start(id_sb[:], id_t.ap())
            # ones column for softmax denominators: mk[:, 1023] is all ones
            nc.sync.dma_start(
                v_sb[:, :, :, D:D + 1].rearrange("p a b c -> p (a b) c"),
                mk_t.ap()[:, None, 1023:1024].broadcast_to([128, NKT * HC, 1]))

            # ---- phase 0: x -> xT via PE transpose, through DRAM scratch ----
            xtc = [dram.tile([C, 512], f32r, name=f"xtc{j}") for j in range(NJ)]
            with tc.tile_pool(name="ph0", bufs=1) as ph0:
                for tt in range(NKT):
                    xload = ph0.tile([128, C], f32r, tag="xload", bufs=2,
                                     name=f"xload{tt}")
                    nc.sync.dma_start(xload[:], x_t.ap()[128 * tt:128 * tt + 128, :])
                    for ct in range(NCT):
                        tps = psum.tile([128, 128], f32, tag="s512", bufs=4,
                                        name=f"tps{tt}_{ct}")
                        nc.tensor.transpose(tps[:].bitcast(f32r),
                                            xload[:, 128 * ct:128 * ct + 128],
                                            id_sb[:])
                        stg = ph0.tile([128, 128], f32r, tag="stg", bufs=4,
                                       name=f"stg{tt}_{ct}")
                        nc.vector.tensor_copy(stg[:], tps[:])
                        nc.sync.dma_start(
                            xtc[tt // 4][128 * ct:128 * ct + 128,
                                         128 * (tt % 4):128 * (tt % 4) + 128],
                            stg[:])

            # ---- phases 1-3 pipelined over chunks j -------------------------
            for j in range(NJ):
                c0 = 512 * j  # column/query range [c0, c0+512)

                # -- phase 1: projections for chunk j --
                xtn = []
                for k in range(NCT):
                    t_ = work.tile([128, 512], f32r, tag="xtn", bufs=8,
                                   name=f"xtn{j}_{k}")
                    nc.sync.dma_start(t_[:], xtc[j][128 * k:128 * k + 128, :])
                    xtn.append(t_)

                qTc = work.tile([128, 4, 512], f32r, tag="qTc", bufs=2,
                                name=f"qTc{j}")

                for g in range(8):
                    wqk_g = work.tile([128, NCT, 128], f32r, tag="wqkg", bufs=2,
                                      name=f"wqkg{j}_{g}")
                    nc.sync.dma_start(
                        wqk_g[:],
                        wqk_t.ap()[:, 128 * g:128 * g + 128]
                        .rearrange("(kt p) m -> p kt m", p=128))
                    pqk = psum.tile([128, 512], f32, tag="s512", bufs=4,
                                    name=f"pqk{j}_{g}")
                    for k in range(NCT):
                        nc.tensor.matmul(pqk[:], wqk_g[:, k, :], xtn[k][:],
                                         start=(k == 0), stop=(k == NCT - 1))
                    if g < 4:
                        nc.vector.tensor_copy(qTc[:, g, :], pqk[:])
                    else:
                        nc.vector.tensor_copy(kT_sb[:, g - 4, c0:c0 + 512], pqk[:])

                # -- RoPE on chunk j (q groups then k groups) --
                css = work.tile([128, 512], f32r, tag="css", bufs=1, name=f"css{j}")
                nc.sync.dma_start(css[:], cs_t.ap()[:, c0:c0 + 512])
                sns = work.tile([128, 512], f32r, tag="sns", bufs=1, name=f"sns{j}")
                nc.sync.dma_start(sns[:], sn_t.ap()[:, c0:c0 + 512])

                for src, pi in [(qTc, 0), (qTc, 1), (kT_sb, 0), (kT_sb, 1)]:
                    if src is qTc:
                        x1 = qTc[:, 2 * pi, :]
                        x2 = qTc[:, 2 * pi + 1, :]
                    else:
                        x1 = kT_sb[:, 2 * pi, c0:c0 + 512]
                        x2 = kT_sb[:, 2 * pi + 1, c0:c0 + 512]
                    t1 = work.tile([128, 512], f32r, tag="rt", bufs=4,
                                   name=f"t1_{j}_{pi}")
                    nc.vector.tensor_tensor(t1[:], x1, css[:], MUL)
                    t2 = work.tile([128, 512], f32r, tag="rt", bufs=4,
                                   name=f"t2_{j}_{pi}")
                    nc.vector.tensor_tensor(t2[:], x1, sns[:], MUL)
                    t3 = work.tile([128, 512], f32r, tag="rt", bufs=4,
                                   name=f"t3_{j}_{pi}")
                    nc.vector.tensor_tensor(t3[:], x2, sns[:], MUL)
                    nc.vector.tensor_tensor(x1, t1[:], t3[:], SUB)
                    t4 = work.tile([128, 512], f32r, tag="rt", bufs=4,
                                   name=f"t4_{j}_{pi}")
                    nc.vector.tensor_tensor(t4[:], x2, css[:], MUL)
                    nc.vector.tensor_tensor(x2, t4[:], t2[:], ADD)

                # -- v projection for chunk j (4 key tiles) --
                for tt4 in range(4):
                    kt = 4 * j + tt4
                    pv = psum.tile([128, 512], f32, tag="s512", bufs=4,
                                   name=f"pv{j}_{tt4}")
                    for k in range(NCT):
                        nc.tensor.matmul(pv[:], xtn[k][:, 128 * tt4:128 * tt4 + 128],
                                         wv_sb[:, k, :],
                                         start=(k == 0), stop=(k == NCT - 1))
                    for h in range(HC):
                        nc.vector.tensor_copy(v_sb[:, kt, h, 0:D],
                                              pv[:, D * h:D * h + D])

                # -- phase 2: attention for chunk j --
                aot = work.tile([128, 4, 512], f32r, tag="aot", bufs=2,
                                name=f"aot{j}")
                nk = 4 * (j + 1)
                for hg in range(2):
                    ge, go = 2 * hg, 2 * hg + 1
                    av = [psum.tile([D + 1, 512], f32, tag="av", bufs=4,
                                    name=f"av{j}_{hg}_{h4}") for h4 in range(4)]
                    for i in range(nk):
                        for h4 in range(4):
                            h = 4 * hg + h4
                            r0 = 32 * h4
                            sps = psum.tile([128, 512], f32, tag="s512", bufs=4,
                                            name=f"sps{j}_{hg}_{i}_{h4}")
                            tp = (r0, 0)
                            nc.tensor.matmul(
                                sps[:],
                                kT_sb[r0:r0 + 32, ge, 128 * i:128 * i + 128],
                                qTc[r0:r0 + 32, ge, :],
                                start=True, stop=False, tile_position=tp)
                            nc.tensor.matmul(
                                sps[:],
                                kT_sb[r0:r0 + 32, go, 128 * i:128 * i + 128],
                                qTc[r0:r0 + 32, go, :],
                                start=False, stop=True, tile_position=tp)
                            pt = work.tile([128, 512], f32r, tag="pt", bufs=6,
                                           name=f"pt{j}_{hg}_{i}_{h4}")
                            nc.scalar.activation(pt[:], sps[:], EXP)
                            off = 128 * i - 512 * j
                            if off >= 0:
                                w = off + 128
                                nc.vector.tensor_tensor(
                                    pt[:, 0:w], pt[:, 0:w],
                                    mk_sb[:, 512 - off:512 - off + w], MUL)
                            nc.tensor.matmul(av[h4][:], v_sb[:, i, h, :], pt[:],
                                             start=(i == 0), stop=(i == nk - 1))
                    for h4 in range(4):
                        h = 4 * hg + h4
                        recip = work.tile([1, 512], f32, tag="recip", bufs=2,
                                          name=f"rc{j}_{hg}_{h4}")
                        nc.vector.reciprocal(recip[:], av[h4][D:D + 1, :])
                        bc = work.tile([64, 512], f32, tag="bc", bufs=2,
                                       name=f"bc{j}_{hg}_{h4}")
                        nc.gpsimd.partition_broadcast(bc[:], recip[:])
                        nc.vector.tensor_tensor(
                            aot[64 * (h % 2):64 * (h % 2) + 64, h // 2, :],
                            av[h4][0:D, :], bc[:], MUL)

                # -- phase 3: out-projection for chunk j --
                for tt4 in range(4):
                    for cc in range(2):
                        yps = psum.tile([128, 512], f32, tag="s512", bufs=4,
                                        name=f"yps{j}_{tt4}_{cc}")
                        for ct in range(4):
                            nc.tensor.matmul(
                                yps[:],
                                aot[:, ct, 128 * tt4:128 * tt4 + 128],
                                wout_sb[:, ct, 512 * cc:512 * cc + 512],
                                start=(ct == 0), stop=(ct == 3))
                        yst = work.tile([128, 512], f32, tag="yst", bufs=2,
                                        name=f"yst{j}_{tt4}_{cc}")
                        nc.vector.tensor_copy(yst[:], yps[:])
                        nc.sync.dma_start(
                            y_t.ap()[c0 + 128 * tt4:c0 + 128 * tt4 + 128,
                                     512 * cc:512 * cc + 512],
                            yst[:])

    nc.compile()
    return nc


def _host_inputs(x, W_qkv, W_out):
    """Per-core input dicts (numpy, fp32)."""
    x = np.ascontiguousarray(np.asarray(x), dtype=np.float32)
    W_qkv = np.ascontiguousarray(np.asarray(W_qkv), dtype=np.float32)
    W_out = np.ascontiguousarray(np.asarray(W_out), dtype=np.float32)

    inv_freq = (1.0 / (THETA ** (np.arange(0, D, 2, dtype=np.float32) / D))).astype(np.float32)
    freqs = np.arange(T, dtype=np.float32)[:, None] * inv_freq[None, :]  # [T, 32]
    cs = np.tile(np.cos(freqs).T.astype(np.float32), (4, 1))  # [128, T]
    sn = np.tile(np.sin(freqs).T.astype(np.float32), (4, 1))
    kk = np.arange(128)[:, None]
    cc = np.arange(1024)[None, :]
    mk = (cc >= kk + 512).astype(np.float32)
    ident = np.eye(128, dtype=np.float32)

    in_maps = []
    for core in range(N_CORES):
        b, hg = core // 2, core % 2
        h0 = HC * hg  # first global head
        # permuted q/k columns: groups of 128 = (4 heads) x (32 even-or-odd dims)
        cols = []
        for s in range(2):  # 0=q, 1=k
            for quad in range(2):          # heads [4*quad, 4*quad+4)
                for par in range(2):       # 0=even dims, 1=odd dims
                    for hh in range(4):
                        hglob = h0 + 4 * quad + hh
                        for i_ in range(32):
                            cols.append(s * (H * D) + hglob * D + 2 * i_ + par)
        cols = np.asarray(cols)
        wqk = W_qkv[:, cols].copy()
        wqk[:, 0:512] *= np.float32(1.0 / np.sqrt(D))  # fold score scale into Wq
        wv = W_qkv[:, 2 * H * D + h0 * D: 2 * H * D + (h0 + HC) * D].copy()
        wout = W_out[h0 * D:(h0 + HC) * D, :].copy()
        in_maps.append({
            "x": x[b], "wqk": wqk, "wv": wv, "wout": wout,
            "cs": cs, "sn": sn, "mk": mk, "ident": ident,
        })
    return in_maps


def _get_runtime():
    """Compile once; return a cached sharded jitted callable + metadata."""
    if "rt" in _CACHE:
        return _CACHE["rt"]
    import jax
    import numpy as _np
    from jax.sharding import Mesh, PartitionSpec
    from jax.experimental.shard_map import shard_map
    import concourse.mybir as mybir
    from concourse import bass2jax

    nc = _build_program()
    bass2jax.install_neuronx_cc_hook()

    partition_name = (nc.partition_id_tensor.name
                      if nc.partition_id_tensor else None)
    in_names, out_names, out_avals, zero_outs = [], [], [], []
    for alloc in nc.m.functions[0].allocations:
        if not isinstance(mybir_alloc := alloc, mybir.MemoryLocationSet):
            continue
        name = alloc.memorylocations[0].name
        if alloc.kind == "ExternalInput":
            if name != partition_name:
                in_names.append(name)
        elif alloc.kind == "ExternalOutput":
            np_dt = mybir.dt.np(alloc.dtype)
            out_names.append(name)
            out_avals.append(jax.core.ShapedArray(tuple(alloc.tensor_shape), np_dt))
            zero_outs.append(_np.zeros(tuple(alloc.tensor_shape), np_dt))

    n_params = len(in_names)
    n_outs = len(out_names)
    all_in_names = list(in_names) + list(out_names)
    if partition_name is not None:
        all_in_names.append(partition_name)
    donate = tuple(range(n_params, n_params + n_outs))

    def _body(*args):
        operands = list(args)
        if partition_name is not None:
            operands.append(bass2jax.partition_id_tensor())
        outs = bass2jax._bass_exec_p.bind(
            *operands,
            out_avals=tuple(out_avals),
            in_names=tuple(all_in_names),
            out_names=tuple(out_names),
            lowering_input_output_aliases=(),
            sim_require_finite=True,
            sim_require_nnan=True,
            nc=nc,
        )
        return tuple(outs)

    devices = jax.devices()[:N_CORES]
    mesh = Mesh(np.asarray(devices), ("core",))
    in_specs = (PartitionSpec("core"),) * (n_params + n_outs)
    out_specs = (PartitionSpec("core"),) * n_outs
    fn = jax.jit(
        shard_map(_body, mesh=mesh, in_specs=in_specs, out_specs=out_specs,
                  check_rep=False),
        donate_argnums=donate, keep_unused=True)

    rt = dict(fn=fn, in_names=in_names, out_names=out_names,
              zero_outs=zero_outs, mesh=mesh)
    _CACHE["rt"] = rt
    return rt


def _run(in_maps):
    rt = _get_runtime()
    concat_in = [np.concatenate([np.asarray(in_maps[c][n]) for c in range(N_CORES)],
                                axis=0) for n in rt["in_names"]]
    concat_zeros = [np.zeros((N_CORES * z.shape[0], *z.shape[1:]), z.dtype)
                    for z in rt["zero_outs"]]
    out_arrs = rt["fn"](*concat_in, *concat_zeros)
    (y_name,) = rt["out_names"]
    y_all = np.asarray(out_arrs[0]).reshape(N_CORES, T, C)
    return y_all


def kernel(x, W_qkv, W_out):
    in_maps = _host_inputs(x, W_qkv, W_out)
    y_all = _run(in_maps)
    y = np.empty((B, T, C), dtype=np.float32)
    for b in range(B):
        y[b] = y_all[2 * b] + y_all[2 * b + 1]
    return y
